# revision 31
# baseline (speedup 1.0000x reference)
"""IterativeNormalization (whitening) Bass kernel for 8 Trainium2 NeuronCores.

Strategy (data-parallel over batch, per sharding hint):
  - Host shards x on B: each of 8 cores gets (4,48,48,512) -> (9216, 512),
    pre-converted to bf16 and pre-padded with a ones column per group:
    x ships as [9216, 4, 129] bf16 (col 128 = 1.0). This halves input DMA
    and removes all on-device fp32->bf16 converts / ones memsets.
  - Pass 1 (per core): stream 4-chunk (512-row) DMA loads; per 128-row chunk
    accumulate per-group raw second moment M2[g] (128x129, incl. channel sums
    via the ones column) in PSUM, and PE-transpose each group tile (bf16,
    reusing the loaded weights) into an SBUF-resident xT (c,n) copy.
  - AllReduce the packed stats (4 x 128 x 129 fp32, ~264KB) across 8 cores.
  - cov = (1-eps)/(N-1) * (M2 - N mu mu^T) + eps*I, trace, sig = cov/tr,
    3 Newton-Schulz iterations (tiny fp32 128x128 matmuls, replicated).
  - Pass 2: out = xT^T @ (gamma*rsq-scaled P) per (group, chunk) in bf16,
    bias (beta - W @ mu) added during the PSUM drain, streamed out fp32 in
    4-chunk (1MB) DMA stores.
"""

import sys

if "/opt/trn_rl_repo" not in sys.path:
    sys.path.insert(0, "/opt/trn_rl_repo")

import numpy as np

import concourse.bass as bass
import concourse.bacc as bacc
import concourse.tile as tile
from concourse import mybir
from concourse.alu_op_type import AluOpType
from concourse.bass_utils import run_bass_kernel_spmd
from concourse.bass_interp import get_hw_module

N_CORES = 8
B, H, W_DIM, C = 32, 48, 48, 512
G, M = 4, 128
N_TOT = B * H * W_DIM          # 73728
B_LOC = B // N_CORES           # 4
N_LOC = B_LOC * H * W_DIM      # 9216
CHUNKS = N_LOC // 128          # 72
SUB = 4                        # chunks per DMA load/store
LOADS = CHUNKS // SUB          # 18
EPS = 1e-7
NS_ITERS = 3
F32 = mybir.dt.float32
BF16 = mybir.dt.bfloat16

_CACHE: dict = {}


def _bcast_ap(src: bass.AP, parts: int, free_steps) -> bass.AP:
    """Broadcast a source AP across `parts` partitions with given free dims."""
    return bass.AP(tensor=src.tensor, offset=src.offset, ap=[[0, parts]] + free_steps)


def _ptile(tc, shape, dtype, name):
    return tc._singles_pool.tile(shape, dtype, tag=name, name=name)


def _kernel_body(tc, x_d, gamma_d, beta_d, eye_d, out_d, collective=True, rep=0):
    nc = tc.nc
    a_const = (1.0 - EPS) / (N_TOT - 1.0)
    # outer-product scale: outer = (mu*s1)(mu*s1)^T must equal N*a*mu*mu^T,
    # where mu = s / N. So s1 applied to raw channel sums s is sqrt(N*a)/N.
    s1 = float(np.sqrt(N_TOT * a_const) / N_TOT)

    # x_d: [N_LOC, G, 129] bf16; row n = (l*SUB + s)*128 + p
    x_v = x_d.rearrange("(l s p) g w -> l p s g w", s=SUB, p=128)
    out_v = out_d.rearrange("(l s p) c -> l p s c", s=SUB, p=128)

    # PSUM drains: only DVE (Act is reserved for the lone Rsqrt so its
    # activation-function table never reloads; GPSIMD/Pool cannot read PSUM)
    def drain_copy(idx, out, in_):
        nc.vector.tensor_copy(out=out, in_=in_)

    # ---------------- persistent tiles ----------------
    singles_cm = tc.tile_pool(name="singles", bufs=1)
    tc._singles_pool = singles_cm.__enter__()
    xT = _ptile(tc, [128, G * N_LOC], BF16, "xT")        # 72KB/partition
    xT_v = xT.rearrange("p (g n) -> p g n", g=G)
    eye_sb = _ptile(tc, [128, 128], F32, "eye_sb")
    nc.sync.dma_start(out=eye_sb, in_=eye_d)
    eye_bf = _ptile(tc, [128, 128], BF16, "eye_bf")
    nc.vector.tensor_copy(out=eye_bf, in_=eye_sb)
    eyepack = _ptile(tc, [128, G * 128], F32, "eyepack")
    for g in range(G):
        nc.gpsimd.tensor_copy(out=eyepack[:, g * 128:(g + 1) * 128], in_=eye_sb)
    gamma_bc = _ptile(tc, [128, C], F32, "gamma_bc")
    nc.gpsimd.dma_start(out=gamma_bc, in_=_bcast_ap(gamma_d, 128, [[1, C]]))
    beta_col = _ptile(tc, [128, G], F32, "beta_col")
    nc.gpsimd.dma_start(
        out=beta_col,
        in_=bass.AP(tensor=beta_d.tensor, offset=beta_d.offset, ap=[[1, 128], [128, G]]),
    )
    gamma_col = _ptile(tc, [128, G], F32, "gamma_col")
    nc.gpsimd.dma_start(
        out=gamma_col,
        in_=bass.AP(tensor=gamma_d.tensor, offset=gamma_d.offset, ap=[[1, 128], [128, G]]),
    )
    ones1 = _ptile(tc, [128, 1], F32, "ones1")
    nc.vector.memset(ones1, 1.0)
    ones_row = _ptile(tc, [1, 128], F32, "ones_row")
    nc.vector.memset(ones_row, 1.0)
    ones_row_bf = _ptile(tc, [1, 128], BF16, "ones_row_bf")
    nc.vector.memset(ones_row_bf, 1.0)
    stats_sb = _ptile(tc, [128, G, 129], F32, "stats_sb")
    ar_sb = _ptile(tc, [128, G, 129], F32, "ar_sb")
    # 1.5*I pack, precomputed during pass 1 (constant)
    pt15 = _ptile(tc, [128, G * 128], F32, "pt15")
    nc.gpsimd.tensor_scalar_mul(pt15, eyepack, 1.5)
    # scratch for warming the Act engine's Sqrt table off the critical path
    warm = _ptile(tc, [1, 1], F32, "warm")

    with tc.tile_pool(name="dram", bufs=1, space="DRAM") as drampool:
        cc_in = drampool.tile([128, G, 129], F32, name="cc_in")
        cc_out = drampool.tile([128, G, 129], F32, name="cc_out", addr_space="Shared")

        # ===== pass 1a: stream x + cov stats only (DMA runs flat out) =====
        # ===== pass 1b: PE transposes + xT drains, overlapping AllReduce ====
        with (
            tc.tile_pool(name="xpool", bufs=1) as xpool,
            tc.tile_pool(name="m2pool", bufs=1, space="PSUM") as m2pool,
            tc.tile_pool(name="tpool", bufs=3, space="PSUM") as tpool,
        ):
            xst = xpool.tile([128, LOADS, SUB, G, 129], BF16, name="xst")
            m2ps = [
                m2pool.tile([128, 129], F32, tag=f"m2_{g}", name=f"m2_{g}")
                for g in range(G)
            ]
            for l in range(LOADS):
                nc.sync.dma_start(out=xst[:, l], in_=x_v[l])
                for s in range(SUB):
                    i = l * SUB + s
                    for g in range(G):
                        nc.tensor.matmul(
                            m2ps[g][:, :],
                            lhsT=xst[:, l, s, g, :128],
                            rhs=xst[:, l, s, g, :],
                            start=(i == 0),
                            stop=(i == CHUNKS - 1),
                            skip_group_check=True,
                        )
            # drain stats PSUM -> SBUF -> DRAM for the collective. On Act: DVE
            # is busy with xT drains and would delay the all-reduce launch.
            for g in range(G):
                nc.scalar.copy(out=stats_sb[:, g, :], in_=m2ps[g][:, :])
            nc.sync.dma_start(out=cc_in, in_=stats_sb)

            # all-reduce launches now; transposes below overlap its latency
            if collective:
                nc.gpsimd.collective_compute(
                    "AllReduce",
                    AluOpType.add,
                    replica_groups=[list(range(N_CORES))],
                    ins=[cc_in.opt()],
                    outs=[cc_out.opt()],
                )
            else:
                nc.gpsimd.dma_start(out=cc_out.opt(), in_=cc_in.opt())
            nc.sync.dma_start(out=ar_sb, in_=cc_out)

            for l in range(LOADS):
                for s in range(SUB):
                    i = l * SUB + s
                    t_ps = tpool.tile([128, G, 128], BF16)
                    for g in range(G):
                        nc.tensor.transpose(
                            t_ps[:, g], in_=xst[:, l, s, g, :128], identity=eye_bf,
                        )
                    nc.vector.tensor_copy(
                        out=xT_v[:, :, i * 128:(i + 1) * 128], in_=t_ps)
            # warm the Sqrt table after the Act drains, hidden in AR latency
            nc.scalar.sqrt(warm, ones_row[0:1, 0:1])

        # ================= Newton-Schulz (replicated) =================
        with (
            tc.tile_pool(name="nssb", bufs=1) as nssb,
            tc.tile_pool(name="nsps", bufs=1, space="PSUM") as nsps,
            tc.tile_pool(name="smps", bufs=2, space="PSUM") as smps,
        ):
            GP = G * 128
            # --- two parallel chains off ar_sb ---
            # X chain: mu -> murow -> outer -> cov
            mu_bf = _ptile(tc, [128, G], BF16, "mu_bf")
            nc.gpsimd.tensor_scalar_mul(mu_bf, ar_sb[:, :, 128], 1.0 / N_TOT)
            mu_sc = _ptile(tc, [128, G], F32, "mu_sc")
            nc.gpsimd.tensor_scalar_mul(mu_sc, ar_sb[:, :, 128], s1)
            mu_scb = _ptile(tc, [128, G], BF16, "mu_scb")
            nc.gpsimd.tensor_scalar_mul(mu_scb, ar_sb[:, :, 128], s1)

            murow_ps = smps.tile([1, G * 128], BF16, tag="smallb")
            for g in range(G):
                nc.tensor.transpose(
                    murow_ps[0:1, g * 128:(g + 1) * 128],
                    in_=mu_scb[:, g:g + 1], identity=eye_bf,
                )
            murow_sb = _ptile(tc, [1, G * 128], BF16, "murow_sb")
            nc.vector.tensor_copy(out=murow_sb, in_=murow_ps)

            outer_ps = nsps.tile([128, GP], F32, tag="mm")
            for g in range(G):
                sl = slice(g * 128, (g + 1) * 128)
                nc.tensor.matmul(
                    outer_ps[:, sl],
                    lhsT=murow_sb[0:1, sl], rhs=murow_sb[0:1, sl], start=True, stop=True,
                )
            # cov = a*M2 - outer   (eps*I term dropped: 1e-7 << diag ~1)
            cov = _ptile(tc, [128, GP], F32, "cov")
            nc.vector.scalar_tensor_tensor(
                out=cov.rearrange("p (g w) -> p g w", g=G),
                in0=ar_sb[:, :, :128], scalar=a_const, op0=AluOpType.mult,
                in1=outer_ps.rearrange("p (g w) -> p g w", g=G), op1=AluOpType.subtract,
            )
            # Y chain: trace directly from raw stats:
            # tr_g = a*sum_m M2[m,m] - |mu_sc_g|^2
            diag = _ptile(tc, [128, GP], F32, "diag")
            nc.gpsimd.tensor_mul(
                diag.rearrange("p (g w) -> p g w", g=G), ar_sb[:, :, :128],
                eyepack.rearrange("p (g w) -> p g w", g=G),
            )
            diagv = _ptile(tc, [128, G], F32, "diagv")
            nc.vector.tensor_reduce(
                diagv, diag.rearrange("p (g w) -> p g w", g=G),
                axis=mybir.AxisListType.X, op=AluOpType.add,
            )
            sc_ps = smps.tile([1, 2 * G], F32, tag="small")
            nc.tensor.matmul(sc_ps[0:1, 0:G], lhsT=ones1, rhs=diagv, start=True, stop=True)
            for g in range(G):
                nc.tensor.matmul(
                    sc_ps[0:1, G + g:G + g + 1],
                    lhsT=mu_sc[:, g:g + 1], rhs=mu_sc[:, g:g + 1], start=True, stop=True,
                )
            sc_sb = _ptile(tc, [1, 2 * G], F32, "sc_sb")
            nc.vector.tensor_copy(out=sc_sb, in_=sc_ps)
            tr_row = _ptile(tc, [1, G], F32, "tr_row")
            nc.vector.scalar_tensor_tensor(
                out=tr_row, in0=sc_sb[0:1, 0:G], scalar=a_const, op0=AluOpType.mult,
                in1=sc_sb[0:1, G:2 * G], op1=AluOpType.subtract,
            )
            rtr_row = _ptile(tc, [1, G], F32, "rtr_row")
            nc.vector.reciprocal(rtr_row, tr_row)
            rsq_row = _ptile(tc, [1, G], F32, "rsq_row")
            nc.scalar.sqrt(rsq_row, rtr_row)
            # broadcast rtr/rsq down partitions via K=1 matmul with ones_row
            rb_ps = smps.tile([128, 2 * G], F32, tag="small")
            nc.tensor.matmul(rb_ps[:, 0:G], lhsT=ones_row, rhs=rtr_row, start=True, stop=True)
            nc.tensor.matmul(rb_ps[:, G:2 * G], lhsT=ones_row, rhs=rsq_row, start=True, stop=True)
            rb_sb = _ptile(tc, [128, 2 * G], F32, "rb_sb")
            nc.vector.tensor_copy(out=rb_sb, in_=rb_ps)
            rtr_b = rb_sb[:, 0:G]
            rsq_b = rb_sb[:, G:2 * G]
            # --- join ---
            sig_bf = _ptile(tc, [128, GP], BF16, "sig_bf")
            gsq = _ptile(tc, [128, GP], F32, "gsq")
            gcr = _ptile(tc, [128, G], F32, "gcr")
            nc.gpsimd.tensor_mul(gcr, gamma_col, rsq_b)
            for g in range(G):
                sl = slice(g * 128, (g + 1) * 128)
                nc.vector.tensor_scalar_mul(sig_bf[:, sl], cov[:, sl], rtr_b[:, g:g + 1])
                # gamma * rsq column scale (overlaps NS; consumed at wmat)
                nc.gpsimd.tensor_scalar_mul(gsq[:, sl], gamma_bc[:, sl], rsq_b[:, g:g + 1])
            # P = 1.5*I - 0.5*sig ; then 2 full NS iterations (all bf16 on PE)
            P = _ptile(tc, [128, GP], BF16, "P")
            nc.vector.scalar_tensor_tensor(
                out=P, in0=sig_bf, scalar=-0.5, op0=AluOpType.mult, in1=pt15, op1=AluOpType.add,
            )
            for it in range(NS_ITERS - 1):
                # P^3 sig = (P@P) @ (P@sig): one PE pass with shared weights P,
                # the two PSUM drains run in parallel on DVE and Act
                ab_ps = nsps.tile([128, 2, GP], F32, tag="mm")
                for g in range(G):
                    sl = slice(g * 128, (g + 1) * 128)
                    nc.tensor.matmul(ab_ps[:, 0, sl], lhsT=P[:, sl], rhs=P[:, sl], start=True, stop=True)
                    nc.tensor.matmul(ab_ps[:, 1, sl], lhsT=P[:, sl], rhs=sig_bf[:, sl], start=True, stop=True)
                a_sb = nssb.tile([128, GP], BF16, tag="scratch")
                nc.vector.tensor_copy(out=a_sb, in_=ab_ps[:, 0])
                b_sb = nssb.tile([128, GP], BF16, tag="scratchb")
                nc.scalar.copy(out=b_sb, in_=ab_ps[:, 1])
                t3_ps = nsps.tile([128, GP], F32, tag="mm3")
                for g in range(G):
                    sl = slice(g * 128, (g + 1) * 128)
                    nc.tensor.matmul(t3_ps[:, sl], lhsT=a_sb[:, sl], rhs=b_sb[:, sl], start=True, stop=True)
                # pt = 1.5*P off the critical path (Pool), P update on DVE
                pt = nssb.tile([128, GP], F32, tag="scratch2")
                nc.gpsimd.tensor_scalar_mul(pt, P, 1.5)
                nc.vector.scalar_tensor_tensor(
                    out=P, in0=t3_ps, scalar=-0.5, op0=AluOpType.mult, in1=pt, op1=AluOpType.add,
                )
            # W = P * (gamma*rsq) column scale; symmetric P. Straight to bf16.
            wmat_bf = _ptile(tc, [128, GP], BF16, "wmat_bf")
            nc.vector.tensor_mul(wmat_bf, gsq, P)
            # bias = beta - W^T @ mu = beta - (gamma*rsq)_col ⊙ (P @ mu)
            vp_ps = smps.tile([128, G], F32, tag="small")
            for g in range(G):
                nc.tensor.matmul(
                    vp_ps[:, g:g + 1],
                    lhsT=P[:, g * 128:(g + 1) * 128],
                    rhs=mu_bf[:, g:g + 1], start=True, stop=True,
                )
            v_col = _ptile(tc, [128, G], F32, "v_col")
            nc.vector.tensor_mul(v_col, vp_ps, gcr)
            bias_col = _ptile(tc, [128, G], F32, "bias_col")
            nc.gpsimd.tensor_sub(bias_col, beta_col, v_col)
            brow_ps = smps.tile([1, C], F32, tag="small")
            for g in range(G):
                nc.tensor.transpose(
                    brow_ps[0:1, g * 128:(g + 1) * 128],
                    in_=bias_col[:, g:g + 1], identity=eye_sb,
                )
            biasrow_bf = _ptile(tc, [1, C], BF16, "biasrow_bf")
            nc.vector.tensor_copy(out=biasrow_bf, in_=brow_ps)

        # ================= pass 2: whitening apply =================
        with (
            tc.tile_pool(name="opool", bufs=3) as opool,
            tc.tile_pool(name="ops", bufs=4, space="PSUM") as opsp,
        ):
            for l in range(LOADS):
                o_sb = opool.tile([128, SUB, C], F32)
                for s in range(SUB):
                    i = l * SUB + s
                    o_ps = opsp.tile([128, C], F32)
                    # bias row first (start=True zeroes the whole 2KB PSUM
                    # bank, so the full-width matmul must come first), then
                    # the 4 per-group whitening matmuls accumulate on top
                    nc.tensor.matmul(
                        o_ps, lhsT=ones_row_bf, rhs=biasrow_bf,
                        start=True, stop=False, skip_group_check=True,
                    )
                    for g in range(G):
                        sl = slice(g * 128, (g + 1) * 128)
                        nc.tensor.matmul(
                            o_ps[:, sl],
                            lhsT=xT_v[:, g, i * 128:(i + 1) * 128],
                            rhs=wmat_bf[:, sl], start=False, stop=(g == G - 1),
                            skip_group_check=True,
                        )
                    drain_copy(i, o_sb[:, s], o_ps)
                nc.sync.dma_start(out=out_v[l], in_=o_sb)
    singles_cm.__exit__(None, None, None)


def build_nc(reps: int = 1, collective: bool = True, num_devices: int = N_CORES):
    nc = bacc.Bacc("TRN2", target_bir_lowering=False, debug=False, num_devices=num_devices)
    x_d = nc.dram_tensor("x", [N_LOC, G, 129], BF16, kind="ExternalInput").ap()
    gamma_d = nc.dram_tensor("gamma", [C], F32, kind="ExternalInput").ap()
    beta_d = nc.dram_tensor("beta", [C], F32, kind="ExternalInput").ap()
    eye_d = nc.dram_tensor("eye", [128, 128], F32, kind="ExternalInput").ap()
    out_d = nc.dram_tensor("out", [N_LOC, C], F32, kind="ExternalOutput").ap()
    with tile.TileContext(nc) as tc:
        for rep in range(reps):
            _kernel_body(tc, x_d, gamma_d, beta_d, eye_d, out_d,
                         collective=collective, rep=rep)
    nc.compile()
    return nc


def make_in_maps(x: np.ndarray, gamma: np.ndarray, beta: np.ndarray):
    bf16 = mybir.dt.np(BF16)
    x = np.asarray(x, dtype=np.float32).reshape(B, H * W_DIM, C)
    gamma = np.asarray(gamma, dtype=np.float32).reshape(C)
    beta = np.asarray(beta, dtype=np.float32).reshape(C)
    eye = np.eye(128, dtype=np.float32)
    in_maps = []
    for i in range(N_CORES):
        xs = x[i * B_LOC:(i + 1) * B_LOC].reshape(N_LOC, G, M).astype(bf16)
        xp = np.empty((N_LOC, G, M + 1), dtype=bf16)
        xp[:, :, :M] = xs
        xp[:, :, M] = np.float32(1.0)
        in_maps.append({"x": xp, "gamma": gamma, "beta": beta, "eye": eye})
    return in_maps


def kernel(x, gamma, beta):
    if "nc" not in _CACHE:
        nc = build_nc()
        nc.m = get_hw_module(nc.m)
        _CACHE["nc"] = nc
    nc = _CACHE["nc"]
    in_maps = make_in_maps(x, gamma, beta)
    res = run_bass_kernel_spmd(nc, in_maps, list(range(N_CORES)))
    out = np.concatenate(
        [res.results[i]["out"].reshape(B_LOC, H, W_DIM, C) for i in range(N_CORES)],
        axis=0,
    )
    return out.astype(np.float32)


if __name__ == "__main__":
    rng = np.random.default_rng(0)
    x = rng.standard_normal((B, H, W_DIM, C), dtype=np.float32)
    gamma = rng.random((1, 1, 1, C), dtype=np.float32)
    beta = rng.standard_normal((1, 1, 1, C), dtype=np.float32)
    out = kernel(x, gamma, beta)
    print("out", out.shape, out.dtype, float(np.abs(out).max()))
